# revision 1
# baseline (speedup 1.0000x reference)
"""BiAttention kernel for Trainium2 (8 NeuronCores, data-parallel over batch).

Computation (per batch b):
  energy[s, h] = tanh( enc[s, :] @ W_e.T + (hidden[b] @ W_h.T + attn_b) )
  att[s]       = energy[s, :] @ v
  out[b, s]    = softmax(att)[s]

Device strategy (per core, 2 batches each):
  - Host pre-transposes encoder_outputs to k-major [B, 2H, S] so the device
    streams it straight into the PE array as the matmul moving operand.
  - energy^T computed as [h=128 partitions, tokens] so the (hidden@W_h + b)
    term folds into the Tanh activation's per-partition bias.
  - v-reduction is a second matmul (v stationary, tanh output streaming).
  - Softmax over the full 8192-token row per batch with a constant shift
    (exact: |att| <= sum|v| <= 128 keeps exp finite); exp + per-partition
    sums fused in ACT; partition sums via SWDGE hops or tiny matmuls.
"""

import os
import sys
import numpy as np
from contextlib import ExitStack

if "/opt/trn_rl_repo" not in sys.path:
    sys.path.insert(0, "/opt/trn_rl_repo")

from concourse import bass, bacc, tile, mybir
from concourse.bass_utils import run_bass_kernel_spmd

B, S, H = 16, 8192, 256
NCORES = 8
BPC = B // NCORES          # batches per core
GT = int(os.environ.get("K_GT", "1024"))  # tokens per DMA group
ST = 512                   # tokens per compute subtile / psum bank
NSI = GT // ST
NG = S // GT               # DMA groups per batch
NR = S // ST               # rows in the per-batch attention tile (16)
NKC = 4                    # k chunks (2H=512 -> 4x128)
NHC = 2                    # h chunks (H=256 -> 2x128)

F32 = mybir.dt.float32
F32R = mybir.dt.float32r
AF = mybir.ActivationFunctionType
ALU = mybir.AluOpType
AX = mybir.AxisListType

_CACHE = {}

LAST_RESULT = None
LAST_IN_MAPS = None


def _build(reps=1):
    key = ("nc", reps)
    if key in _CACHE:
        return _CACHE[key]

    nc = bacc.Bacc("TRN2", target_bir_lowering=False, debug=False,
                   num_devices=NCORES)

    encT_d = nc.dram_tensor("encT", [BPC, NKC, 128, S], F32R, kind="ExternalInput").ap()
    wT_d = nc.dram_tensor("wT", [NKC, 128, H], F32R, kind="ExternalInput").ap()
    biasT_d = nc.dram_tensor("biasT", [BPC, NHC, 128, 1], F32, kind="ExternalInput").ap()
    vT_d = nc.dram_tensor("vT", [NHC, 128, 1], F32R, kind="ExternalInput").ap()
    out_d = nc.dram_tensor("out", [BPC, S], F32, kind="ExternalOutput").ap()

    with tile.TileContext(nc) as tc, ExitStack() as ctx:
        wpool = ctx.enter_context(tc.tile_pool(name="wpool", bufs=1))
        cpool = ctx.enter_context(tc.tile_pool(name="cpool", bufs=1))
        enc_pool = ctx.enter_context(tc.tile_pool(
            name="enc", bufs=int(os.environ.get("K_ENCBUFS", "8"))))
        tanh_pool = ctx.enter_context(tc.tile_pool(name="tanh", bufs=int(os.environ.get("K_TANH", "8"))))
        att_pool = ctx.enter_context(tc.tile_pool(name="att", bufs=int(os.environ.get("K_ATT", "2"))))
        stat_pool = ctx.enter_context(tc.tile_pool(name="stat", bufs=4))
        out_pool = ctx.enter_context(tc.tile_pool(name="outp", bufs=int(os.environ.get("K_OUTP", "2"))))
        epsum_pool = ctx.enter_context(tc.tile_pool(
            name="epsum", bufs=int(os.environ.get("K_EPSUM", "6")), space="PSUM"))
        apsum_pool = ctx.enter_context(tc.tile_pool(
            name="apsum", bufs=int(os.environ.get("K_APSUM", "2")), space="PSUM"))

        # --- preamble: w[0] first so the first matmul can start, then the
        # first enc group, then the remaining weights ---
        w_all = wpool.tile([128, NKC, H], F32R, tag="w_all")
        w_sb = [w_all[:, kc, :] for kc in range(NKC)]
        nc.sync.dma_start(w_all[:, 0, :], wT_d[0])
        chunks0 = []
        for kc in range(NKC):
            c = enc_pool.tile([128, GT], F32R, tag="enc", name=f"c0_{kc}")
            nc.sync.dma_start(c[:], encT_d[0, kc, :, 0:GT])
            chunks0.append(c)
        nc.sync.dma_start(w_all[:, 1:, :],
                          wT_d[1:].rearrange("kc p h -> p kc h"))
        bias_all = wpool.tile([128, BPC * NHC], F32, tag="bias_all")
        nc.gpsimd.dma_start(bias_all[:],
                            biasT_d.rearrange("b hc p x -> p (b hc x)"))
        bias_sb = [[bias_all[:, b * NHC + hc:b * NHC + hc + 1]
                    for hc in range(NHC)] for b in range(BPC)]
        v_all = wpool.tile([128, NHC], F32R, tag="v_all")
        nc.gpsimd.dma_start(v_all[:], vT_d.rearrange("hc p x -> p (hc x)"))
        v_sb = [v_all[:, hc:hc + 1] for hc in range(NHC)]

        ones4 = cpool.tile([1, 4], F32, tag="ones4")
        nc.gpsimd.memset(ones4[:], 1.0)
        ones128 = cpool.tile([1, 128], F32, tag="ones128")
        nc.gpsimd.memset(ones128[:], 1.0)
        mask128 = cpool.tile([128, 1], F32, tag="mask128")
        nc.gpsimd.memset(mask128[:], 0.0)
        nc.gpsimd.dma_start(mask128[0:128:32, :], ones4[:])
        # Constant softmax shift: out = exp(att - 40) / sum(exp(att - 40)).
        # Shift-invariant exactly; |att| <= sum|v| <= 128 and exp(128-40)
        # stays finite in fp32, so no overflow for any input to this model.
        cneg = cpool.tile([128, 1], F32, tag="cneg")
        nc.gpsimd.memset(cneg[:], -40.0)

        _vr_ctr = [0]

        def v_reduce(tanhs, att_tile, r):
            # att[r, :] = sum_h v[h] * tanh[h, :] -- two accumulating matmuls
            # with v stationary, then one DVE copy of the [1, ST] psum row
            # straight into the batch tile. Row r lives at partition 32*(r%4)
            # (a DVE-legal start partition), column block r//4.
            _vr_ctr[0] += 1
            ap = apsum_pool.tile([1, ST], F32, tag="ap",
                                 name=f"ap_{_vr_ctr[0]}")
            for hc in range(NHC):
                nc.tensor.matmul(ap[:], v_sb[hc], tanhs[hc][:],
                                 start=(hc == 0), stop=(hc == NHC - 1))
            q, cb = 32 * (r % 4), r // 4
            nc.vector.tensor_copy(
                att_tile[q:q + 1, cb * ST:(cb + 1) * ST], ap[:])

        def emit_exp_cb(att_all, exp_sb, sums4, cb):
            # exp of column block cb (rows 4cb..4cb+3) as soon as its rows
            # are in att_all; per-partition partial sums land in sums4[:, cb]
            nc.scalar.activation(exp_sb[:, cb * ST:(cb + 1) * ST],
                                 att_all[:, cb * ST:(cb + 1) * ST], AF.Exp,
                                 bias=cneg[:], accum_out=sums4[:, cb:cb + 1])

        def emit_tail(att_all, exp_sb, sums4, b, last):
            # softmax tail for batch b (exp already emitted per column block):
            # partition-sum and broadcast via SWDGE (earlier batches) or tiny
            # PE matmuls (final batch, when PE is idle), scale on DVE.
            sums = stat_pool.tile([128, 1], F32, tag="sums", name=f"sums{b}_{rep}")
            nc.vector.reduce_sum(sums[:], sums4[:], axis=AX.X)

            inv128 = stat_pool.tile([128, 1], F32, tag="inv128",
                                    name=f"inv128_{b}_{rep}")
            if not last:
                srow = stat_pool.tile([1, 4], F32, tag="srow", name=f"sr{b}_{rep}")
                nc.gpsimd.dma_start(srow[:], sums[0:128:32, :])
                tot_sb = stat_pool.tile([1, 1], F32, tag="tot", name=f"to{b}_{rep}")
                nc.vector.reduce_sum(tot_sb[:], srow[:], axis=AX.X)
                inv = stat_pool.tile([1, 1], F32, tag="inv", name=f"iv{b}_{rep}")
                nc.vector.reciprocal(inv[:], tot_sb[:])
                invrow = stat_pool.tile([1, 4], F32, tag="invrow",
                                        name=f"ir{b}_{rep}")
                nc.vector.tensor_scalar_mul(invrow[:], ones4[:], inv[:])
                nc.gpsimd.memset(inv128[:], 0.0)
                nc.gpsimd.dma_start(inv128[0:128:32, :], invrow[:])
            else:
                # att_all was memset, so unused partitions hold finite values
                # (exp(-40)*2048); mask128 zeroes them out of the total.
                tot_ps = apsum_pool.tile([1, 1], F32, tag="ap",
                                          name=f"tot{b}_{rep}")
                nc.tensor.matmul(tot_ps[:], sums[:], mask128[:],
                                 start=True, stop=True)
                tot_sb = stat_pool.tile([1, 1], F32, tag="tot", name=f"to{b}_{rep}")
                nc.vector.tensor_copy(tot_sb[:], tot_ps[:])
                inv = stat_pool.tile([1, 1], F32, tag="inv", name=f"iv{b}_{rep}")
                nc.vector.reciprocal(inv[:], tot_sb[:])
                inv_ps = apsum_pool.tile([128, 1], F32, tag="ap",
                                          name=f"ib{b}_{rep}")
                nc.tensor.matmul(inv_ps[:], ones128[:], inv[:],
                                 start=True, stop=True)
                nc.vector.tensor_copy(inv128[:], inv_ps[:])

            res = out_pool.tile([128, 4 * ST], F32, tag="res", name=f"res{b}_{rep}")
            nc.vector.tensor_scalar_mul(res[:], exp_sb[:], inv128[:])
            # out[b, 2048*cb + 512*q + u] = res[32*q, 512*cb + u]
            eng = nc.sync if last else nc.gpsimd
            eng.dma_start(
                out_d[b].rearrange("(cb q u) -> q cb u", cb=4, q=4, u=ST),
                res[0:128:32, :].rearrange("q (cb u) -> q cb u", u=ST))

        vq = []  # pending v-reduces, emitted two subtiles late
        pending_tail = None

        def flush_vq(n):
            while len(vq) > n:
                tanhs_, att_, rv = vq.pop(0)
                v_reduce(tanhs_, att_, rv)
                if rv % 4 == 3:
                    emit_exp_cb(att_, vq_exp[0], vq_exp[1], rv // 4)

        for rep, b in [(rp, bb) for rp in range(reps) for bb in range(BPC)]:
            # rows r=0..15 at (partition 32*(r%4), column block r//4)
            att_all = att_pool.tile([128, 4 * ST], F32, tag="att", name=f"att_{rep}_{b}")
            nc.gpsimd.memset(att_all[:], 0.0)
            exp_sb = out_pool.tile([128, 4 * ST], F32, tag="exp",
                                   name=f"exp{rep}_{b}")
            sums4 = stat_pool.tile([128, 4], F32, tag="sums4",
                                   name=f"sums4_{rep}_{b}")
            vq_exp = (exp_sb, sums4)
            last_batch = (rep == reps - 1 and b == BPC - 1)
            for g in range(NG):
                split_last = last_batch and g == NG - 1 and \
                    os.environ.get("K_SPLITLAST", "0") == "1"
                if rep == 0 and b == 0 and g == 0:
                    chunks = chunks0
                elif not split_last:
                    chunks = []
                    for kc in range(NKC):
                        c = enc_pool.tile([128, GT], F32R, tag="enc",
                                          name=f"c{rep}_{b}_{g}_{kc}")
                        nc.sync.dma_start(
                            c[:], encT_d[b, kc, :, g * GT:(g + 1) * GT])
                        chunks.append(c)
                for si in range(NSI):
                    r = g * NSI + si
                    if split_last:
                        # final group: per-subtile 512-token loads so the last
                        # compute chain starts half a group earlier
                        chunks = []
                        t0 = g * GT + si * ST
                        for kc in range(NKC):
                            c = enc_pool.tile([128, GT], F32R, tag="enc",
                                              name=f"cl{rep}_{b}_{si}_{kc}")
                            nc.sync.dma_start(
                                c[:, 0:ST], encT_d[b, kc, :, t0:t0 + ST])
                            chunks.append(c)
                    epsums = [epsum_pool.tile([128, ST], F32, tag="ep",
                                              name=f"ep_{rep}_{b}_{r}_{i}")
                              for i in range(NHC)]
                    tanhs = []
                    for hc in range(NHC):
                        for kc in range(NKC):
                            nc.tensor.matmul(
                                epsums[hc][:],
                                w_sb[kc][:, hc * 128:(hc + 1) * 128],
                                chunks[kc][:, 0:ST] if split_last else
                                chunks[kc][:, si * ST:(si + 1) * ST],
                                start=(kc == 0), stop=(kc == NKC - 1))
                        th = tanh_pool.tile([128, ST], F32R, tag="th")
                        nc.scalar.activation(th[:], epsums[hc][:], AF.Tanh,
                                             bias=bias_sb[b][hc])
                        tanhs.append(th)
                    vq.append((tanhs, att_all, r))
                    flush_vq(int(os.environ.get("K_VQ", "2")))
                if pending_tail is not None and g == 1:
                    # emit the previous batch's remaining softmax tail here so
                    # it queues behind only two groups of this batch's work
                    emit_tail(*pending_tail, last=False)
                    pending_tail = None
            # flush remaining subtiles of this batch
            flush_vq(0)
            if rep < reps - 1 or b < BPC - 1:
                pending_tail = (att_all, exp_sb, sums4, b)
                if b == BPC - 1:
                    # next rep re-enters at g==1 of its first batch
                    pass
            else:
                emit_tail(att_all, exp_sb, sums4, b, last=True)


    nc.compile()
    _CACHE[key] = nc
    return nc


def kernel(hidden, encoder_outputs, attn_w, attn_b, v):
    global LAST_RESULT
    hidden = np.asarray(hidden, dtype=np.float32)
    encoder_outputs = np.asarray(encoder_outputs, dtype=np.float32)
    attn_w = np.asarray(attn_w, dtype=np.float32)
    attn_b = np.asarray(attn_b, dtype=np.float32)
    v = np.asarray(v, dtype=np.float32)

    # host-side marshaling (tiny except the one-time layout change of enc)
    encT = np.ascontiguousarray(encoder_outputs.transpose(0, 2, 1))  # [B, 2H, S]
    W_h = attn_w[:, :H]
    bias_hb = hidden[:, 0, :] @ W_h.T + attn_b                       # [B, H]
    wT = np.ascontiguousarray(attn_w[:, H:].T).reshape(NKC, 128, H)  # [4,128,256]
    vT = np.ascontiguousarray(v).reshape(NHC, 128, 1)

    nc = _build()
    in_maps = []
    for c in range(NCORES):
        sl = slice(BPC * c, BPC * (c + 1))
        in_maps.append({
            "encT": encT[sl].reshape(BPC, NKC, 128, S),
            "wT": wT,
            "biasT": np.ascontiguousarray(bias_hb[sl]).reshape(BPC, NHC, 128, 1),
            "vT": vT,
        })

    trace = bool(os.environ.get("KERNEL_TRACE"))
    if trace:
        try:
            from antenv.axon_hooks import get_axon_ntff_profile_hook  # noqa: F401
        except ImportError:
            trace = False
    res = run_bass_kernel_spmd(
        nc, in_maps, core_ids=list(range(NCORES)), trace=trace)
    LAST_RESULT = res
    globals()["LAST_IN_MAPS"] = in_maps
    out = np.concatenate(
        [res.results[c]["out"].reshape(BPC, S) for c in range(NCORES)], axis=0)
    return out.reshape(B, 1, S).astype(np.float32)


if __name__ == "__main__":
    rng = np.random.default_rng(0)
    hid = rng.standard_normal((B, 1, H), dtype=np.float32)
    enc = rng.standard_normal((B, S, 2 * H), dtype=np.float32)
    aw = rng.standard_normal((H, 3 * H), dtype=np.float32) / np.sqrt(3 * H)
    ab = rng.standard_normal(H, dtype=np.float32) * 0.01
    vv = rng.random(H, dtype=np.float32)
    out = kernel(hid, enc, aw, ab, vv)
    print(out.shape, out.sum(axis=-1))

